# revision 9
# baseline (speedup 1.0000x reference)
"""Trainium2 Bass kernel for nn_Attention_9242769622327.

Math: the reference computes
    qkv = x @ W1.T ; q,k,v = split(qkv)
    score = softmax(k^T v / 4, axis=-1)            # rows sum to 1
    attn  = softmax(einsum('bhnk,bhkc->bhnk', q/4, score), axis=-1)
          = softmax(q/4)                           # sum_c score == 1, k/v dead
    out   = attn @ W2.T
so only the q-projection (first E rows of W1), a per-head (64-wide) softmax,
and the output projection are needed.

Distribution: pure data-parallel over the 32768 = B*S rows; each of the 8
cores handles 4096 rows with the full weights. No collectives.

Precision: EVERY matmul runs in fp8-e4m3 DoubleRow (2 fp8 MACs per PE cell
per cycle -> half the matmul instructions of bf16, and no PE dtype-mode
switches anywhere).  Feeding attn itself to an fp8 mm2 would cost ~2.5%
output error (over the 2% gate), but attn rows sum to exactly 1 per 64-wide
head, so
    out[j,m] = K[j] + sum_n W2T[n,j] * d[n,m],   d = attn - 1/64,
    K[j]     = sum_n W2T[n,j] / 64   (constant, added on the HOST in fp32).
The deviation d is ~4x smaller than attn, so fp8 noise on it (and on an
fp8 W2) lands at ~0.6% each on the output; measured total 1.34% rel err.
Scales: W1q and W2T are host-prescaled by 32 (entries std 1/32 -> 1), the
head-sum selector holds 1/64 so the head-sum PSUM is s/64 and its
reciprocal 64/s; then at = u*(64/s) = 64*attn, d8 = fp8(at - 1) = 64*d,
and the host divides the gathered output by 32*64 = 2048 before adding K.

On-chip layout is fully transposed (features on partitions, rows on the free
dim) so no on-chip transposes are needed anywhere:
    qT[n,m]  = sum_k W1qT[k,n] * xT[k,m]          (PE, fp8 DR, K=256/MM)
    u        = exp(qT/128)                        (ACT, PSUM->SBUF fp16)
    u8       = fp8(u)                             (DVE, one whole-strip copy)
    s[g,m]   = sum_{n in head g} u8[n,m] / 64     (PE fp8-DR w/ 1/64 selector)
    rcp      = 1/s = 64/head-sum                  (DVE reciprocal_approx_fast)
    rbf[n,m] = rcp[head(n),m]                     (DMA partition-broadcast,
                                                   16 x [64,MS] SBUF->SBUF)
    at       = u * rbf                            (DVE, one whole-strip mul)
    d8       = fp8(at - 1)                        (DVE, one whole-strip add)
    outT[j,m]= sum_n (32*W2T)[n,j] * d8[n,m]      (PE, fp8 DR)

The rcp broadcast runs on the DMA engines (16 x 64KB SBUF->SBUF descriptors
per stripe) instead of PE broadcast-matmuls: v2 measured those fp16 matmuls
at 400-570ns each (PE dtype-mode switch + LDWEIGHTS exposure) plus PSUM-bank
pacing stalls.  With the broadcast off the PE, the softmax chain for stripe
ms-1 (at -> d8, two merged DVE ops, ~5us) finishes before mm1(ms) does, so a
single-stage software pipeline suffices:
  PE:  [32 mm1(ms)] [32 mm2(ms-1)] [4 hs(ms)]     = 68 DR MMs ~14.5us
  DVE: [at(ms-1)] [d8(ms-1)] [u8(ms)] [o-drains j4-7 (ms-1)] [recip(ms)]
  ACT: [8 exp(ms)] [o-drains j0-3 (ms-1)] [rcp f32->f16 copy(ms)]
Whole-strip [128, 4096] DVE/ACT ops amortize the ~320ns per-op engine
overhead measured on [128,512] ops.  PSUM: 3 q banks, 4 out, 1 head-sum.
w2 DMAs are deferred behind the stripe-0/1 x + w1 loads; 8 throwaway
matmuls on memset scratch warm the PE HAM clock gate during that window.
x/w1/w2 are host-packed so every DMA tile is one contiguous DRAM block.
"""

import sys

sys.path.insert(0, "/opt/trn_rl_repo")

import numpy as np
import ml_dtypes

import concourse.bass as bass
import concourse.bacc as bacc
import concourse.tile as tile
from concourse import mybir
from concourse.bass_utils import run_bass_kernel_spmd

BF16 = mybir.dt.float16  # fp16: same PE rate as bf16, 10-bit mantissa
FP8 = mybir.dt.float8e4
F32 = mybir.dt.float32
AF = mybir.ActivationFunctionType
DR = mybir.MatmulPerfMode.DoubleRow

N_CORES = 8
B, S, E = 4, 8192, 1024
HEADS, HEAD_DIM = 16, 64
M_TOTAL = B * S                # 32768
M_CORE = M_TOTAL // N_CORES    # 4096 rows per core
MS = 512                       # m-stripe width (moving free dim / PSUM bank)
N_STRIPES = M_CORE // MS       # 8
KP = E // 256                  # 4 DoubleRow contraction pair-chunks
NC_ = E // 128                 # 8 feature chunks
W_SCALE = 32.0                 # pre-scale on W1q / W2T before fp8 quantization
OUT_SCALE = W_SCALE * 64.0     # host divides gathered output by this

_BF = np.float16
_F8 = ml_dtypes.float8_e4m3fn


def build_nc() -> bass.Bass:
    nc = bacc.Bacc("TRN2", debug=False)

    # x/w1/w2 are pre-packed on host so every DMA tile is one contiguous
    # block (1-2KB per-partition lines instead of 512B strided runs)
    xt8 = nc.dram_tensor("xt8", [KP, N_STRIPES, 128, 2 * MS], FP8, kind="ExternalInput")
    w18 = nc.dram_tensor("w18", [KP, 128, 2 * E], FP8, kind="ExternalInput")
    w28 = nc.dram_tensor("w28", [KP, 128, 2 * E], FP8, kind="ExternalInput")
    sel8 = nc.dram_tensor("sel8", [128, KP * 2 * HEADS], FP8, kind="ExternalInput")
    outT = nc.dram_tensor("outT", [E, M_CORE], BF16, kind="ExternalOutput")

    with tile.TileContext(nc) as tc:
        with (
            tc.tile_pool(name="weights", bufs=1) as wpool,
            tc.tile_pool(name="xt", bufs=N_STRIPES) as xpool,
            tc.tile_pool(name="u", bufs=2) as upool,
            tc.tile_pool(name="u8", bufs=2) as u8pool,
            tc.tile_pool(name="at", bufs=2) as apool,
            tc.tile_pool(name="d8", bufs=2) as d8pool,
            tc.tile_pool(name="rbf", bufs=2) as rpool,
            tc.tile_pool(name="small", bufs=3) as spool,
            tc.tile_pool(name="dscr", bufs=3) as dpool,
            tc.tile_pool(name="ostage", bufs=8) as opool,
            tc.tile_pool(name="ps_q", bufs=3, space="PSUM") as psq,
            tc.tile_pool(name="ps_o", bufs=4, space="PSUM") as pso,
            tc.tile_pool(name="ps_s", bufs=1, space="PSUM") as pss,
        ):
            # Warm the PE's HAM clock gate with throwaway matmuls on memset
            # scratch while the first weight/x DMAs are in flight, so the
            # first real matmuls run at 2.4 GHz instead of 1.2.
            warm_sb = wpool.tile([128, MS], BF16, name="warm_sb")
            nc.gpsimd.memset(warm_sb[:], 0.0)
            warm_ps = psq.tile([128, MS], F32, tag="q", name="warm_ps")
            for _ in range(8):
                nc.tensor.matmul(
                    warm_ps[:], warm_sb[:, 0:128], warm_sb[:], start=True, stop=True
                )

            # Stripe-0-critical loads first: w1 pair-chunks interleaved with
            # stripe-0 x pair-chunks, then the tiny sel8.  w2 is deferred
            # until after stripe 1's x loads (first read ~1.5 stripes in).
            w1_t = []
            xt0 = []
            for t in range(KP):
                w = wpool.tile([128, 2, E], FP8, tag=f"w1_{t}", name=f"w1t{t}")
                nc.sync.dma_start(w[:], w18[t, :, :])
                w1_t.append(w)
                tx = xpool.tile([128, 2, MS], FP8, tag=f"xt_{t}", name=f"xt0_{t}")
                nc.sync.dma_start(tx[:], xt8[t, 0, :, :])
                xt0.append(tx)
            sel8_t = wpool.tile([128, KP, 2, HEADS], FP8, name="sel8_t")
            nc.sync.dma_start(
                sel8_t[:],
                sel8[:, :].rearrange("p (t two h) -> p t two h", two=2, h=HEADS),
            )

            w2_k = [
                wpool.tile([128, 2, E], FP8, tag=f"w2_{t}", name=f"w2k{t}")
                for t in range(KP)
            ]

            # ---- software pipeline state ----
            u_hist = {}    # stripe -> [128, NC_, MS] fp16 u strip
            rbf_hist = {}  # stripe -> [128, NC_, MS] fp16 broadcast-rcp strip

            for it in range(N_STRIPES + 1):
                cur = it if it < N_STRIPES else None   # mm1/u8/hs stripe
                prv = it - 1 if it >= 1 else None      # at/d8/mm2 stripe

                # ---- x DMA for the current stripe ----
                if cur is not None and cur > 0:
                    xt_k = []
                    for t in range(KP):
                        tx = xpool.tile(
                            [128, 2, MS], FP8, tag=f"xt_{t}", name=f"xt{cur}_{t}"
                        )
                        nc.sync.dma_start(tx[:], xt8[t, cur, :, :])
                        xt_k.append(tx)
                elif cur == 0:
                    xt_k = xt0
                if it == 1:
                    for t in range(KP):
                        nc.sync.dma_start(w2_k[t][:], w28[t, :, :])

                # ---- softmax chain of stripe prv: two whole-strip DVE ops
                # (emitted first so they are at the head of the DVE queue;
                # they unblock mm2(prv) which the PE reaches ~7us in) ----
                if prv is not None:
                    at_strip = apool.tile(
                        [128, NC_, MS], BF16, tag="at", name="at_strip"
                    )
                    nc.vector.tensor_mul(
                        at_strip[:, :, :].rearrange("p c m -> p (c m)"),
                        u_hist[prv][:, :, :].rearrange("p c m -> p (c m)"),
                        rbf_hist[prv][:, :, :].rearrange("p c m -> p (c m)"),
                    )
                    d8_strip = d8pool.tile(
                        [128, KP, 2, MS], FP8, tag="d8", name="d8_strip"
                    )
                    nc.vector.tensor_scalar_add(
                        d8_strip[:, :, :, :].rearrange("p t i m -> p (t i m)"),
                        at_strip[:, :, :].rearrange("p c m -> p (c m)"),
                        -1.0,
                    )
                    del u_hist[prv]
                    del rbf_hist[prv]

                # ---- mm1(cur): q-projection, 32-MM fp8-DR block + exp ----
                if cur is not None:
                    u_strip = upool.tile([128, NC_, MS], BF16, tag="u", name="u_strip")
                    for ci in range(NC_):
                        q_ps = psq.tile([128, MS], F32, tag="q", name="q_ps")
                        for t in range(KP):
                            nc.tensor.matmul(
                                q_ps[:],
                                w1_t[t][:, :, ci * 128:(ci + 1) * 128],
                                xt_k[t][:],
                                start=(t == 0),
                                stop=(t == KP - 1),
                                perf_mode=DR,
                            )
                        nc.scalar.activation(
                            u_strip[:, ci, :], q_ps[:], AF.Exp,
                            scale=1.0 / (4.0 * W_SCALE),
                        )
                    u_hist[cur] = u_strip

                    # fp8 copy of the whole u strip for the DR head-sum: the
                    # [t, i] pair-interleave order IS ci order, so one DVE op
                    # does all 8 chunks
                    u8_strip = u8pool.tile(
                        [128, KP, 2, MS], FP8, tag="u8", name="u8_strip"
                    )
                    nc.vector.tensor_scalar_mul(
                        u8_strip[:, :, :, :].rearrange("p t i m -> p (t i m)"),
                        u_strip[:, :, :].rearrange("p c m -> p (c m)"),
                        1.0,
                    )

                # ---- mm2(prv): output projection, 32-MM fp8-DR block ----
                if prv is not None:
                    for j in range(NC_):
                        o_ps = pso.tile([128, MS], F32, tag="o", name="o_ps")
                        for t in range(KP):
                            nc.tensor.matmul(
                                o_ps[:],
                                w2_k[t][:, :, j * 128:(j + 1) * 128],
                                d8_strip[:, t, :, :],
                                start=(t == 0),
                                stop=(t == KP - 1),
                                perf_mode=DR,
                            )
                        o_t = opool.tile([128, MS], BF16, tag="ost", name="o_t")
                        if j < 4:
                            nc.scalar.copy(o_t[:], o_ps[:])
                        else:
                            nc.vector.tensor_scalar_mul(o_t[:], o_ps[:], 1.0)
                        nc.sync.dma_start(
                            outT[j * 128:(j + 1) * 128, prv * MS:(prv + 1) * MS],
                            o_t[:],
                        )

                # ---- head sums (4-MM fp8-DR block) + reciprocal (cur) ----
                if cur is not None:
                    s_ps = pss.tile([128, MS], F32, tag="s", name="s_ps")
                    for t in range(KP):
                        nc.tensor.matmul(
                            s_ps[0:HEADS, :],
                            sel8_t[:, t, :, :],
                            u8_strip[:, t, :, :],
                            start=(t == 0),
                            stop=(t == KP - 1),
                            perf_mode=DR,
                        )
                    rcp32 = spool.tile([HEADS, MS], F32, tag="rcp32", name="rcp32")
                    nc.vector.reciprocal_approx_fast(rcp32[:], s_ps[0:HEADS, :])
                    rcp16 = spool.tile([HEADS, MS], BF16, tag="rcp16", name="rcp16")
                    nc.scalar.copy(rcp16[:], rcp32[:])

                    # broadcast rcp16 head rows to the full 128-partition
                    # feature layout on the DMA engines.  A step-0 partition
                    # source is only legal from DRAM, so bounce the tiny
                    # [16, MS] tile through a DRAM scratch tile first, then
                    # issue 16 x [64, MS] broadcast reads.
                    rcp_d = dpool.tile(
                        [HEADS, MS], BF16, tag="rcpd", name="rcp_d",
                        space="DRAM",
                    )
                    nc.sync.dma_start(rcp_d[:], rcp16[:])
                    rbf_strip = rpool.tile(
                        [128, NC_, MS], BF16, tag="rbf", name="rbf_strip"
                    )
                    for ci in range(NC_):
                        for b in range(2):
                            h = 2 * ci + b
                            nc.sync.dma_start(
                                rbf_strip[64 * b:64 * b + 64, ci, :],
                                rcp_d[h:h + 1, :].to_broadcast([64, MS]),
                            )
                    rbf_hist[cur] = rbf_strip
    nc.compile()
    return nc


_NC_CACHE = None
LAST_RESULT = None


def _ensure_ntff_hook():
    """bass_utils' axon trace path needs antenv.axon_hooks, which this
    container's antenv lacks. Provide it + register the ctypes NTFF hook."""
    import types

    try:
        from antenv.axon_hooks import get_axon_ntff_profile_hook  # noqa: F401
        return True
    except ImportError:
        pass
    try:
        import antenv
        from trn_agent_boot.trn_boot import _ntff_profile_via_ctypes

        m = types.ModuleType("antenv.axon_hooks")
        state = {"hook": None}
        m.set_axon_ntff_profile_hook = lambda h: state.__setitem__("hook", h)
        m.get_axon_ntff_profile_hook = lambda: state["hook"]
        sys.modules["antenv.axon_hooks"] = m
        antenv.axon_hooks = m
        m.set_axon_ntff_profile_hook(
            _ntff_profile_via_ctypes("/opt/axon/libaxon_pjrt.so")
        )
        return True
    except Exception as e:  # pragma: no cover
        print(f"ntff hook injection failed: {e}")
        return False


def _selectors():
    # head index of global feature n is n // 64; pair-chunk t group i covers
    # chunk ci = 2t+i, i.e. heads 2ci (partitions 0..63) and 2ci+1 (64..127).
    # Entries are 1/64 (exact in fp8) so the head-sum PSUM holds s/64.
    sel8 = np.zeros((128, KP, 2, HEADS), np.float32)
    for t in range(KP):
        for i in range(2):
            ci = 2 * t + i
            sel8[:64, t, i, 2 * ci] = 1.0 / 64.0
            sel8[64:, t, i, 2 * ci + 1] = 1.0 / 64.0
    return np.ascontiguousarray(sel8.reshape(128, KP * 2 * HEADS)).astype(_F8)


def _pack_pairs(wT):
    """[k, n] -> [KP, 128, 2*n] with k = 256t + 128i + p pair interleave."""
    n = wT.shape[1]
    return np.ascontiguousarray(
        wT.reshape(KP, 2, 128, n).transpose(0, 2, 1, 3).reshape(KP, 128, 2 * n)
    )


def kernel(x, W1, W2, heads, trace=False):
    global _NC_CACHE, LAST_RESULT
    x = np.asarray(x, dtype=np.float32)
    W1 = np.asarray(W1, dtype=np.float32)
    W2 = np.asarray(W2, dtype=np.float32)

    X = x.reshape(M_TOTAL, E)
    X8T = np.ascontiguousarray(X.T).astype(_F8)           # [E, M_TOTAL]
    w18p = _pack_pairs((W1[:E, :] * W_SCALE).T.astype(_F8))   # q-proj weights
    w28p = _pack_pairs((W2.T * W_SCALE).astype(_F8))          # [n, j] = W2[j, n]
    sel8 = _selectors()
    # constant part of the output: sum_n W2T[n,j] * (1/64)
    Kj = W2.astype(np.float64).sum(axis=1) / 64.0         # [E], index j

    in_maps = []
    for c in range(N_CORES):
        xt_c = X8T[:, c * M_CORE:(c + 1) * M_CORE]
        xt_p = np.ascontiguousarray(
            xt_c.reshape(KP, 2, 128, N_STRIPES, MS)
            .transpose(0, 3, 2, 1, 4)
            .reshape(KP, N_STRIPES, 128, 2 * MS)
        )
        in_maps.append({"xt8": xt_p, "w18": w18p, "w28": w28p, "sel8": sel8})

    if _NC_CACHE is None:
        _NC_CACHE = build_nc()

    if trace:
        trace = _ensure_ntff_hook()

    res = run_bass_kernel_spmd(_NC_CACHE, in_maps, list(range(N_CORES)), trace=trace)
    LAST_RESULT = res

    OT = np.concatenate(
        [np.asarray(res.results[c]["outT"]).astype(np.float32) for c in range(N_CORES)],
        axis=1,
    )
    out = OT.T * np.float32(1.0 / OUT_SCALE) + Kj.astype(np.float32)[None, :]
    return np.ascontiguousarray(out).reshape(B, S, E)
